# revision 50
# baseline (speedup 1.0000x reference)
"""Trainium2 Bass kernel for nn_AdjacencyMatrixLayer.

Computes, per batch sample b (coordinates x in R^{N x 3}):
    d_ij  = |x_i|^2 - 2 x_i.x_j + |x_j|^2
    A     = exp(-d / sigma^2)
    A     = softmax(A, axis=2) * mask
    out   = A / (sum_j A + 1e-20)

Key structural ideas (v2, on top of the v1 quad kernel):
  * Valid-region truncation: masks are product-of-prefix (valid lengths
    L_b in [N/2, N]); out is zero outside [:L,:L].  Only row-blocks with
    rows < L are computed, at column width W = ceil(L/128)*128, cutting
    ~45% of all engine + DMA work (sum L^2 / (B*N^2) ~ 0.51).
  * Block-major SPMD packing: the work unit is a [128, W] row-block.
    All 8 cores execute ONE identical width-schedule (widths padded so
    each bucket count is divisible by 8); which (sample, row-range) a
    block holds is pure per-core DATA (stationary/moving slices packed
    host-side), so load balance is near-perfect regardless of lengths.
  * One K=22 bf16 matmul per block produces y = -d/sigma^2 - C*(1-v_i*v_j):
    20 hi/lo-split augmented coordinate rows (exact to ~2^-18) + 2 rows
    folding the padding mask (C=144), so masked entries get y - 144.
  * Per block, one of two pointwise schemes, greedily mixed to balance
    the scalar (ACT) and vector (DVE) engines:
      Q (quad):  p = Exp(s*y) on ACT; t = (p+a)*p + accum on DVE stt
                 (1x); out = (t+b)*r on DVE ts (4x).  Minimax quadratic
                 q = p^2+a*p+b ~= K*exp(exp(y)), rel err 5.1e-3.
      E (exact): A = Exp(y) on ACT; q = Exp(A) + accum on ACT;
                 out = q*r on DVE ts (4x).  Exact double exponential;
                 masked entries give A=0, q=1, corrected via the
                 host-provided per-block constant.
    Row renormalization r = 1/(accum + cons) makes the overall scale
    exact; host zero-fills outside [:L,:L].
  * Input is one interleaved stream (per chunk: stationaries then moving
    slices) split over two DMA queues, first chunk tiny so block 0's
    matmul starts right after the ~10us runtime bootstrap.  Output DMA
    alternates SWDGE/HWDGE with strict byte balance (biasing SWDGE
    back-pressures the pipeline through the Pool engine).
  * Measured (8 cores, fast clock state): ~77-79us vs 113.7us for the
    full-area v1 kernel; per-core ACT ~64-67us busy is the wall, with
    DVE ~60us, PE ~46us, out-DMA ~9.2MB.  Device clock state varies
    run-to-run by up to ~18%; compare kernels only back-to-back.
"""

import math
import sys

import numpy as np

for _p in ("/opt/trn_rl_repo", "/root/.axon_site/_ro/trn_rl_repo"):
    if _p not in sys.path:
        sys.path.append(_p)

B, N, D = 16, 2048, 3
NCORES = 8
P = 128            # SBUF partitions / rows per block
MMF = 512          # matmul moving free-dim chunk (= 1 PSUM bank of fp32)
KAUG = 22          # 20 hi/lo aug rows + 2 mask-fold rows
MASKC = 144.0      # mask fold offset: masked entries get y - 144

# minimax fit of (p^2 + a*p + b) / (K * exp(exp(y))) - 1 over y <= 0
QS_S = 0.9943403856229558   # p = exp(QS_S * y)
QS_A = 1.05888673672267     # q = p^2 + QS_A*p + QS_B
QS_B = 1.217950642291432

# engine-time model (ns per moving column / fixed ns per block), measured
# from perfetto traces of this kernel (includes semaphore overheads)
ACT_NS = 1.004      # one ACT pass over [128, W]
DVE_STT_NS = 1.139  # DVE scalar_tensor_tensor (1x)
DVE_TS_NS = 0.401   # DVE tensor_scalar (4x)
Q_ACT_FIX = 350.0   # per-block fixed engine time (instr + semaphores)
Q_DVE_FIX = 700.0
E_ACT_FIX = 1600.0  # charged high: measured ACT leads DVE by ~7us at 800
E_DVE_FIX = 400.0
ACT_FIX0 = 7000.0   # fixed engine overhead (semaphores, drains, tbl load)
DVE_FIX0 = 6000.0
POOL_FIX0 = 9000.0  # cons DMA, per-block qs2 adds, drains
POOL_TS_NS = 13.6   # Pool tensor_scalar ns/col — measured: Q7 software
POOL_TS_FIX = 500.0  # emulation is ~34x slower than DVE ts; never profitable
SWDGE_NSPB = 0.00377  # Pool-side cost of SWDGE output transfers, ns/byte
FORCE_E_TAIL = 4    # last slots DVE-light so the drain is short


def _chunks(nblk):
    """Input-preload chunk boundaries (block ranges).

    Geometric ramp: single-block chunks early so arrival keeps pace with
    PE consumption while the DMA subsystem is cold (measured: halves the
    early PE stall vs. 5-block chunks), growing once transfers stream.
    """
    sizes = [1, 1, 1, 2, 3, 4, 5, 7]
    chunks, b = [], 0
    for s in sizes:
        if b >= nblk:
            break
        chunks.append((b, min(b + s, nblk)))
        b = chunks[-1][1]
    while b < nblk:
        chunks.append((b, min(b + 8, nblk)))
        b = chunks[-1][1]
    return chunks

_CACHE: dict = {}


def _schedule(lengths):
    """Build the common width schedule + per-core block assignment.

    Returns (widths, schemes, assign) where widths[k] is slot k's moving
    width (same for every core), schemes[k] in {"Q", "E"}, and
    assign[c][k] = (sample, row0, width) for core c slot k (dummy slots
    duplicate a real block; their output is ignored).
    """
    blocks = []  # (L, sample, row0)
    for b, L in enumerate(lengths):
        nb = (int(L) + P - 1) // P
        for r in range(nb):
            blocks.append((int(L), b, r * P))
    # sort by L so each slot's 8 blocks have near-equal lengths, then the
    # slot width (max L in the group, 32-aligned) wastes almost nothing
    blocks.sort(key=lambda x: (-x[0], x[1], x[2]))
    while len(blocks) % NCORES:
        blocks.append(blocks[-1])  # dummy duplicate; output ignored

    widths, assign = [], [[] for _ in range(NCORES)]
    for j in range(len(blocks) // NCORES):
        grp = blocks[j * NCORES:(j + 1) * NCORES]
        w = -(-max(g[0] for g in grp) // 32) * 32
        widths.append(w)
        for c in range(NCORES):
            assign[c].append((grp[c][1], grp[c][2], w))

    # put one narrowest block first so the pipeline starts on a small
    # input chunk (rest stays widest-first, ending narrow for the drain)
    order = list(range(len(widths)))
    order = [order[-1]] + order[:-1]
    widths = [widths[i] for i in order]
    for c in range(NCORES):
        assign[c] = [assign[c][i] for i in order]

    # 3-lane balance: per slot pick scheme (Q: exp on ACT + stt on DVE;
    # E: two exps on ACT) and final-scale lane (DVE ts 4x or Pool ts).
    # Steepest-descent over single flips, minimizing the modeled max
    # engine clock.  Pool finals are restricted to E (single-scalar mult,
    # the only form validated on Q7 firmware).
    nslots = len(widths)
    schemes = ["Q"] * nslots
    for k in range(nslots - FORCE_E_TAIL, nslots):
        schemes[k] = "E"
    finals = ["dve"] * nslots
    swdge_est = 0.45 * sum(widths) * P * 2 * SWDGE_NSPB

    def clocks(schemes, finals):
        a, d, p = ACT_FIX0, DVE_FIX0, POOL_FIX0 + swdge_est
        for w, s, f in zip(widths, schemes, finals):
            a += ACT_NS * w + Q_ACT_FIX if s == "Q" \
                else 2 * ACT_NS * w + E_ACT_FIX
            d += DVE_STT_NS * w + Q_DVE_FIX if s == "Q" else E_DVE_FIX
            if f == "dve":
                d += DVE_TS_NS * w
            else:
                p += POOL_TS_NS * w + POOL_TS_FIX
        return max(a, d, p)

    # enumerate E-count; E slots are spread evenly through the schedule
    # so DVE's stt work never back-loads behind an ACT-only stretch.
    # (Pool finals disabled: measured Q7 tensor_scalar is ~34x slower
    # than DVE ts and blocks SWDGE output transfers queued behind it.)
    nbody = nslots - FORCE_E_TAIL
    best = None
    for e in range(0, nbody + 1):
        s2 = ["Q"] * nslots
        for k in range(nbody, nslots):
            s2[k] = "E"
        for i in range(e):
            s2[int((i + 0.5) * nbody / e)] = "E"
        f2 = ["dve"] * nslots
        v = clocks(s2, f2)
        if best is None or v < best[0]:
            best = (v, s2, f2)
    _, schemes, finals = best

    # output queue per block: strict byte balance between SWDGE/HWDGE —
    # biasing bytes toward SWDGE measurably back-pressures the pipeline
    oqueues = []
    sw_b = hw_b = 0
    for w in widths:
        if sw_b <= hw_b:
            sw_b += w * P * 2
            oqueues.append("sw")
        else:
            hw_b += w * P * 2
            oqueues.append("hw")
    return widths, schemes, finals, oqueues, assign


def _build(widths, schemes, finals, oqueues):
    import concourse.bacc as bacc
    import concourse.tile as tile
    from concourse import mybir

    f32 = mybir.dt.float32
    f16 = mybir.dt.float16
    bf16 = mybir.dt.bfloat16
    AT = mybir.AluOpType
    AF = mybir.ActivationFunctionType
    nc = bacc.Bacc(None, target_bir_lowering=False, debug=False)

    nblk = len(widths)
    sumw = sum(widths)

    mov = nc.dram_tensor(
        "mov", [KAUG, sumw + nblk * P], bf16, kind="ExternalInput"
    )
    cons = nc.dram_tensor("cons", [P, nblk], f32, kind="ExternalInput")
    out = nc.dram_tensor("out", [nblk * P, N], f16, kind="ExternalOutput")

    with tile.TileContext(nc) as tc:
        with (
            tc.tile_pool(name="consts", bufs=1) as consts,
            tc.tile_pool(name="work", bufs=12) as workp,
            tc.tile_pool(name="ot", bufs=8) as otp,
            tc.tile_pool(name="small", bufs=20) as smallp,
            tc.tile_pool(name="psum", bufs=2, space="PSUM") as psump,
        ):
            cn_all = consts.tile([P, nblk], f32, tag="cn")

            # single input stream: per chunk, the blocks' stationaries
            # then their moving slices, split over two DMA queues; the
            # first chunk is tiny so block 0's matmul starts early
            chunks = _chunks(nblk)
            chunk_engs = [nc.sync, nc.gpsimd]
            mv_tiles, st_of, mv_of = [], {}, {}
            co = 0
            for ci, (b0, b1) in enumerate(chunks):
                cw = (b1 - b0) * P + sum(widths[b0:b1])
                mt = consts.tile([KAUG, cw], bf16, tag=f"mv{ci}")
                chunk_engs[ci % 2].dma_start(out=mt, in_=mov[:, co:co + cw])
                if ci == 0:
                    nc.gpsimd.dma_start(out=cn_all, in_=cons[:, :])
                mv_tiles.append(mt)
                lo = (b1 - b0) * P
                for k in range(b0, b1):
                    st_of[k] = (ci, (k - b0) * P)
                    mv_of[k] = (ci, lo)
                    lo += widths[k]
                co += cw

            for k in range(nblk):
                W = widths[k]
                ci, so = st_of[k]
                st = mv_tiles[ci][:, so:so + P]
                ci, lo = mv_of[k]
                mv = mv_tiles[ci][:, lo:lo + W]

                ps = psump.tile([P, N], f32)
                for c0 in range(0, W, MMF):
                    cw = min(MMF, W - c0)
                    nc.tensor.matmul(
                        ps[:, c0:c0 + cw], st, mv[:, c0:c0 + cw]
                    )

                qs = smallp.tile([P, 1], f32, tag="qs")
                t = workp.tile([P, N], f16, tag="t")
                if schemes[k] == "Q":
                    # p = exp(s*y); t = (p + a)*p ; qs = sum_j t
                    p = workp.tile([P, N], f16, tag="p")
                    nc.scalar.activation(
                        p[:, :W], ps[:, :W], AF.Exp, scale=QS_S
                    )
                    nc.vector.scalar_tensor_tensor(
                        out=t[:, :W], in0=p[:, :W], scalar=QS_A, in1=p[:, :W],
                        op0=AT.add, op1=AT.mult, accum_out=qs,
                    )
                    cfin = QS_B
                else:
                    # A = exp(y); t = exp(A) ; qs = sum_j t   (exact)
                    a_t = workp.tile([P, N], f16, tag="p")
                    nc.scalar.activation(a_t[:, :W], ps[:, :W], AF.Exp)
                    nc.scalar.activation(
                        t[:, :W], a_t[:, :W], AF.Exp, accum_out=qs
                    )
                    cfin = 0.0

                # qs2 = qs + cons ; r = 1/qs2   (tiny [P,1] ops)
                qs2 = smallp.tile([P, 1], f32, tag="qs2")
                nc.gpsimd.tensor_tensor(
                    out=qs2, in0=qs, in1=cn_all[:, k:k + 1], op=AT.add
                )
                r = smallp.tile([P, 1], f32, tag="r")
                nc.vector.reciprocal(r, qs2)

                # out = (t + cfin) * r
                ot = otp.tile([P, N], f16, tag="ot")
                if finals[k] == "pool":
                    # E scheme only (cfin == 0): single-scalar mult form
                    nc.gpsimd.tensor_scalar(
                        out=ot[:, :W], in0=t[:, :W], scalar1=r,
                        scalar2=None, op0=AT.mult,
                    )
                else:
                    nc.vector.tensor_scalar(
                        out=ot[:, :W], in0=t[:, :W], scalar1=cfin, scalar2=r,
                        op0=AT.add, op1=AT.mult,
                    )
                out_eng = nc.gpsimd if oqueues[k] == "sw" else nc.sync
                out_eng.dma_start(
                    out=out[k * P:(k + 1) * P, :W], in_=ot[:, :W]
                )

    nc.compile()
    return nc


def _lengths_from_masks(masks):
    """Per-sample valid lengths; verifies the product-prefix structure."""
    diag = np.einsum('bii->bi', masks)
    valid = (diag > 0.5).astype(np.float32)
    lengths = valid.sum(axis=1).astype(np.int64)
    n = masks.shape[1]
    pref = (np.arange(n)[None, :] < lengths[:, None]).astype(np.float32)
    if not np.array_equal(valid, pref):
        return None
    if not np.array_equal(masks, valid[:, :, None] * valid[:, None, :]):
        return None
    return lengths, valid


def _prepare(coordinates, masks, sigma):
    """Host-side prep: schedule blocks, pack per-core block-major inputs."""
    import ml_dtypes

    bf = ml_dtypes.bfloat16
    coords = np.ascontiguousarray(np.asarray(coordinates, dtype=np.float32))
    masks = np.asarray(masks, dtype=np.float32)
    sig = float(np.asarray(sigma, dtype=np.float32).reshape(-1)[0])

    res = _lengths_from_masks(masks)
    assert res is not None, "masks are not product-of-prefix form"
    lengths, valid = res
    widths, schemes, finals, oqueues, assign = _schedule(lengths)
    nblk = len(widths)
    sumw = sum(widths)

    norms = np.sum(coords * coords, axis=2, dtype=np.float32)  # [B, N]
    xT = np.swapaxes(coords, 1, 2)                             # [B, 3, N]
    nss = np.float32(-1.0 / (sig * sig))
    aug_x = np.empty((B, 5, N), np.float32)
    aug_x[:, 0:3] = (-2.0 * nss) * xT
    aug_x[:, 3] = nss * norms
    aug_x[:, 4] = nss
    aug_y = np.empty((B, 5, N), np.float32)
    aug_y[:, 0:3] = xT
    aug_y[:, 3] = 1.0
    aug_y[:, 4] = norms

    # hi/lo bf16 split: v = hi + lo, K=5 fp32 -> K=20 bf16 contraction
    xh = aug_x.astype(bf)
    xl = (aug_x - xh.astype(np.float32)).astype(bf)
    yh = aug_y.astype(bf)
    yl = (aug_y - yh.astype(np.float32)).astype(bf)
    # mask fold rows: C*v_i*v_j - C  (exact in bf16: C=144, v in {0,1})
    C = np.float32(MASKC)
    mx = np.stack([C * valid, np.full_like(valid, C)], axis=1).astype(bf)
    my = np.stack([valid, np.full_like(valid, -1.0)], axis=1).astype(bf)
    augx22 = np.concatenate([xh, xl, xh, xl, mx], axis=1)  # [B, 22, N]
    augy22 = np.concatenate([yh, yh, yl, yl, my], axis=1)

    # interleaved input stream layout (must match _build): per chunk,
    # the blocks' [22, 128] stationaries then their moving slices
    chunks = _chunks(nblk)
    in_maps = []
    for c in range(NCORES):
        mov = np.empty((KAUG, sumw + nblk * P), bf)
        cons = np.empty((P, nblk), np.float32)
        co = 0
        for b0, b1 in chunks:
            for k in range(b0, b1):
                b, r0, w = assign[c][k]
                mov[:, co + (k - b0) * P:co + (k - b0 + 1) * P] = (
                    augx22[b][:, r0:r0 + P]
                )
            lo = co + (b1 - b0) * P
            for k in range(b0, b1):
                b, r0, w = assign[c][k]
                mov[:, lo:lo + w] = augy22[b][:, :w]
                lo += w
            co = lo
        for k, (b, r0, w) in enumerate(assign[c]):
            L = float(lengths[b])
            cons[:, k] = QS_B * L if schemes[k] == "Q" else -(w - L)
        in_maps.append({"mov": mov, "cons": cons})
    return in_maps, (lengths, widths, schemes, finals, oqueues, assign)


def _get_nc(widths=None, schemes=None, finals=None, oqueues=None):
    if "nc" not in _CACHE:
        _CACHE["nc"] = _build(widths, schemes, finals, oqueues)
    return _CACHE["nc"]


def kernel(coordinates, masks, sigma):
    import time

    from concourse.bass_utils import run_bass_kernel_spmd

    in_maps, (lengths, widths, schemes, finals, oqueues, assign) = _prepare(
        coordinates, masks, sigma
    )
    nc = _get_nc(widths, schemes, finals, oqueues)
    # the shared trn2 device occasionally reports a transient
    # NRT_EXEC_UNIT_UNRECOVERABLE; it clears on its own within ~a minute
    for attempt in range(4):
        try:
            res = run_bass_kernel_spmd(
                nc, in_maps, core_ids=list(range(NCORES))
            )
            break
        except Exception:  # noqa: BLE001 - retry transient device errors
            if attempt == 3:
                raise
            time.sleep(20 * (attempt + 1))

    full = np.zeros((B, N, N), np.float32)
    for c in range(NCORES):
        buf = res.results[c]["out"]
        for k, (b, r0, w) in enumerate(assign[c]):
            L = int(lengths[b])
            rows = min(P, L - r0)
            if rows <= 0:
                continue
            full[b, r0:r0 + rows, :L] = (
                buf[k * P:k * P + rows, :L].astype(np.float32)
            )
    return full


# revision 51
# speedup vs baseline: 1.0209x; 1.0209x over previous
"""Trainium2 Bass kernel for nn_AdjacencyMatrixLayer.

Computes, per batch sample b (coordinates x in R^{N x 3}):
    d_ij  = |x_i|^2 - 2 x_i.x_j + |x_j|^2
    A     = exp(-d / sigma^2)
    A     = softmax(A, axis=2) * mask
    out   = A / (sum_j A + 1e-20)

Key structural ideas (v2, on top of the v1 quad kernel):
  * Valid-region truncation: masks are product-of-prefix (valid lengths
    L_b in [N/2, N]); out is zero outside [:L,:L].  Only row-blocks with
    rows < L are computed, at column width W = ceil(L/128)*128, cutting
    ~45% of all engine + DMA work (sum L^2 / (B*N^2) ~ 0.51).
  * Block-major SPMD packing: the work unit is a [128, W] row-block.
    All 8 cores execute ONE identical width-schedule (widths padded so
    each bucket count is divisible by 8); which (sample, row-range) a
    block holds is pure per-core DATA (stationary/moving slices packed
    host-side), so load balance is near-perfect regardless of lengths.
  * One K=22 bf16 matmul per block produces y = -d/sigma^2 - C*(1-v_i*v_j):
    20 hi/lo-split augmented coordinate rows (exact to ~2^-18) + 2 rows
    folding the padding mask (C=144), so masked entries get y - 144.
  * Per block, one of two pointwise schemes, greedily mixed to balance
    the scalar (ACT) and vector (DVE) engines:
      Q (quad):  p = Exp(s*y) on ACT; t = (p+a)*p + accum on DVE stt
                 (1x); out = (t+b)*r on DVE ts (4x).  Minimax quadratic
                 q = p^2+a*p+b ~= K*exp(exp(y)), rel err 5.1e-3.
      E (exact): A = Exp(y) on ACT; q = Exp(A) + accum on ACT;
                 out = q*r on DVE ts (4x).  Exact double exponential;
                 masked entries give A=0, q=1, corrected via the
                 host-provided per-block constant.
    Row renormalization r = 1/(accum + cons) makes the overall scale
    exact; host zero-fills outside [:L,:L].
  * Input is one interleaved stream (per chunk: stationaries then moving
    slices) split over two DMA queues, first chunk tiny so block 0's
    matmul starts right after the ~10us runtime bootstrap.  Output DMA
    alternates SWDGE/HWDGE with strict byte balance (biasing SWDGE
    back-pressures the pipeline through the Pool engine).
  * Measured (8 cores, fast clock state): ~77-79us vs 113.7us for the
    full-area v1 kernel; per-core ACT ~64-67us busy is the wall, with
    DVE ~60us, PE ~46us, out-DMA ~9.2MB.  Device clock state varies
    run-to-run by up to ~18%; compare kernels only back-to-back.
"""

import math
import sys

import numpy as np

for _p in ("/opt/trn_rl_repo", "/root/.axon_site/_ro/trn_rl_repo"):
    if _p not in sys.path:
        sys.path.append(_p)

B, N, D = 16, 2048, 3
NCORES = 8
P = 128            # SBUF partitions / rows per block
MMF = 512          # matmul moving free-dim chunk (= 1 PSUM bank of fp32)
KAUG = 22          # 20 hi/lo aug rows + 2 mask-fold rows
MASKC = 144.0      # mask fold offset: masked entries get y - 144

# minimax fit of (p^2 + a*p + b) / (K * exp(exp(y))) - 1 over y <= 0
QS_S = 0.9943403856229558   # p = exp(QS_S * y)
QS_A = 1.05888673672267     # q = p^2 + QS_A*p + QS_B
QS_B = 1.217950642291432

# engine-time model (ns per moving column / fixed ns per block), measured
# from perfetto traces of this kernel (includes semaphore overheads)
ACT_NS = 1.004      # one ACT pass over [128, W]
DVE_STT_NS = 1.139  # DVE scalar_tensor_tensor (1x)
DVE_TS_NS = 0.401   # DVE tensor_scalar (4x)
Q_ACT_FIX = 350.0   # per-block fixed engine time (instr + semaphores)
Q_DVE_FIX = 700.0
E_ACT_FIX = 1600.0  # charged high: measured ACT leads DVE by ~7us at 800
E_DVE_FIX = 400.0
ACT_FIX0 = 7000.0   # fixed engine overhead (semaphores, drains, tbl load)
DVE_FIX0 = 16000.0  # biased +10us: DVE's queue FINISHES ~12us after ACT's
                    # (stt depends on exp), so finish-time balance wants
                    # more work on ACT than busy-time balance suggests
POOL_FIX0 = 9000.0  # cons DMA, per-block qs2 adds, drains
POOL_TS_NS = 13.6   # Pool tensor_scalar ns/col — measured: Q7 software
POOL_TS_FIX = 500.0  # emulation is ~34x slower than DVE ts; never profitable
SWDGE_NSPB = 0.00377  # Pool-side cost of SWDGE output transfers, ns/byte
FORCE_E_TAIL = 4    # last slots DVE-light so the drain is short


def _chunks(nblk):
    """Input-preload chunk boundaries (block ranges).

    Geometric ramp: single-block chunks early so arrival keeps pace with
    PE consumption while the DMA subsystem is cold (measured: halves the
    early PE stall vs. 5-block chunks), growing once transfers stream.
    """
    sizes = [1, 1, 1, 2, 3, 4, 5, 7]
    chunks, b = [], 0
    for s in sizes:
        if b >= nblk:
            break
        chunks.append((b, min(b + s, nblk)))
        b = chunks[-1][1]
    while b < nblk:
        chunks.append((b, min(b + 8, nblk)))
        b = chunks[-1][1]
    return chunks

_CACHE: dict = {}


def _schedule(lengths):
    """Build the common width schedule + per-core block assignment.

    Returns (widths, schemes, assign) where widths[k] is slot k's moving
    width (same for every core), schemes[k] in {"Q", "E"}, and
    assign[c][k] = (sample, row0, width) for core c slot k (dummy slots
    duplicate a real block; their output is ignored).
    """
    blocks = []  # (L, sample, row0)
    for b, L in enumerate(lengths):
        nb = (int(L) + P - 1) // P
        for r in range(nb):
            blocks.append((int(L), b, r * P))
    # sort by L so each slot's 8 blocks have near-equal lengths, then the
    # slot width (max L in the group, 32-aligned) wastes almost nothing
    blocks.sort(key=lambda x: (-x[0], x[1], x[2]))
    while len(blocks) % NCORES:
        blocks.append(blocks[-1])  # dummy duplicate; output ignored

    widths, assign = [], [[] for _ in range(NCORES)]
    for j in range(len(blocks) // NCORES):
        grp = blocks[j * NCORES:(j + 1) * NCORES]
        w = -(-max(g[0] for g in grp) // 32) * 32
        widths.append(w)
        for c in range(NCORES):
            assign[c].append((grp[c][1], grp[c][2], w))

    # put one narrowest block first so the pipeline starts on a small
    # input chunk (rest stays widest-first, ending narrow for the drain)
    order = list(range(len(widths)))
    order = [order[-1]] + order[:-1]
    widths = [widths[i] for i in order]
    for c in range(NCORES):
        assign[c] = [assign[c][i] for i in order]

    # 3-lane balance: per slot pick scheme (Q: exp on ACT + stt on DVE;
    # E: two exps on ACT) and final-scale lane (DVE ts 4x or Pool ts).
    # Steepest-descent over single flips, minimizing the modeled max
    # engine clock.  Pool finals are restricted to E (single-scalar mult,
    # the only form validated on Q7 firmware).
    nslots = len(widths)
    schemes = ["Q"] * nslots
    for k in range(nslots - FORCE_E_TAIL, nslots):
        schemes[k] = "E"
    finals = ["dve"] * nslots
    swdge_est = 0.45 * sum(widths) * P * 2 * SWDGE_NSPB

    def clocks(schemes, finals):
        a, d, p = ACT_FIX0, DVE_FIX0, POOL_FIX0 + swdge_est
        for w, s, f in zip(widths, schemes, finals):
            a += ACT_NS * w + Q_ACT_FIX if s == "Q" \
                else 2 * ACT_NS * w + E_ACT_FIX
            d += DVE_STT_NS * w + Q_DVE_FIX if s == "Q" else E_DVE_FIX
            if f == "dve":
                d += DVE_TS_NS * w
            else:
                p += POOL_TS_NS * w + POOL_TS_FIX
        return max(a, d, p)

    # enumerate E-count; E slots are spread evenly through the schedule
    # so DVE's stt work never back-loads behind an ACT-only stretch.
    # (Pool finals disabled: measured Q7 tensor_scalar is ~34x slower
    # than DVE ts and blocks SWDGE output transfers queued behind it.)
    nbody = nslots - FORCE_E_TAIL
    best = None
    for e in range(0, nbody + 1):
        s2 = ["Q"] * nslots
        for k in range(nbody, nslots):
            s2[k] = "E"
        for i in range(e):
            s2[int((i + 0.5) * nbody / e)] = "E"
        f2 = ["dve"] * nslots
        v = clocks(s2, f2)
        if best is None or v < best[0]:
            best = (v, s2, f2)
    _, schemes, finals = best

    # output queue per block: strict byte balance between SWDGE/HWDGE —
    # biasing bytes toward SWDGE measurably back-pressures the pipeline
    oqueues = []
    sw_b = hw_b = 0
    for w in widths:
        if sw_b <= hw_b:
            sw_b += w * P * 2
            oqueues.append("sw")
        else:
            hw_b += w * P * 2
            oqueues.append("hw")
    return widths, schemes, finals, oqueues, assign


def _build(widths, schemes, finals, oqueues):
    import concourse.bacc as bacc
    import concourse.tile as tile
    from concourse import mybir

    f32 = mybir.dt.float32
    f16 = mybir.dt.float16
    bf16 = mybir.dt.bfloat16
    AT = mybir.AluOpType
    AF = mybir.ActivationFunctionType
    nc = bacc.Bacc(None, target_bir_lowering=False, debug=False)

    nblk = len(widths)
    sumw = sum(widths)

    mov = nc.dram_tensor(
        "mov", [KAUG, sumw + nblk * P], bf16, kind="ExternalInput"
    )
    cons = nc.dram_tensor("cons", [P, nblk], f32, kind="ExternalInput")
    out = nc.dram_tensor("out", [nblk * P, N], f16, kind="ExternalOutput")

    with tile.TileContext(nc) as tc:
        with (
            tc.tile_pool(name="consts", bufs=1) as consts,
            tc.tile_pool(name="work", bufs=12) as workp,
            tc.tile_pool(name="ot", bufs=8) as otp,
            tc.tile_pool(name="small", bufs=20) as smallp,
            tc.tile_pool(name="psum", bufs=2, space="PSUM") as psump,
        ):
            cn_all = consts.tile([P, nblk], f32, tag="cn")

            # single input stream: per chunk, the blocks' stationaries
            # then their moving slices, split over two DMA queues; the
            # first chunk is tiny so block 0's matmul starts early
            chunks = _chunks(nblk)
            chunk_engs = [nc.sync, nc.gpsimd]
            mv_tiles, st_of, mv_of = [], {}, {}
            co = 0
            for ci, (b0, b1) in enumerate(chunks):
                cw = (b1 - b0) * P + sum(widths[b0:b1])
                mt = consts.tile([KAUG, cw], bf16, tag=f"mv{ci}")
                chunk_engs[ci % 2].dma_start(out=mt, in_=mov[:, co:co + cw])
                if ci == 0:
                    nc.gpsimd.dma_start(out=cn_all, in_=cons[:, :])
                mv_tiles.append(mt)
                lo = (b1 - b0) * P
                for k in range(b0, b1):
                    st_of[k] = (ci, (k - b0) * P)
                    mv_of[k] = (ci, lo)
                    lo += widths[k]
                co += cw

            for k in range(nblk):
                W = widths[k]
                ci, so = st_of[k]
                st = mv_tiles[ci][:, so:so + P]
                ci, lo = mv_of[k]
                mv = mv_tiles[ci][:, lo:lo + W]

                ps = psump.tile([P, N], f32)
                for c0 in range(0, W, MMF):
                    cw = min(MMF, W - c0)
                    nc.tensor.matmul(
                        ps[:, c0:c0 + cw], st, mv[:, c0:c0 + cw]
                    )

                qs = smallp.tile([P, 1], f32, tag="qs")
                t = workp.tile([P, N], f16, tag="t")
                if schemes[k] == "Q":
                    # p = exp(s*y); t = (p + a)*p ; qs = sum_j t
                    p = workp.tile([P, N], f16, tag="p")
                    nc.scalar.activation(
                        p[:, :W], ps[:, :W], AF.Exp, scale=QS_S
                    )
                    nc.vector.scalar_tensor_tensor(
                        out=t[:, :W], in0=p[:, :W], scalar=QS_A, in1=p[:, :W],
                        op0=AT.add, op1=AT.mult, accum_out=qs,
                    )
                    cfin = QS_B
                else:
                    # A = exp(y); t = exp(A) ; qs = sum_j t   (exact)
                    a_t = workp.tile([P, N], f16, tag="p")
                    nc.scalar.activation(a_t[:, :W], ps[:, :W], AF.Exp)
                    nc.scalar.activation(
                        t[:, :W], a_t[:, :W], AF.Exp, accum_out=qs
                    )
                    cfin = 0.0

                # qs2 = qs + cons ; r = 1/qs2   (tiny [P,1] ops)
                qs2 = smallp.tile([P, 1], f32, tag="qs2")
                nc.gpsimd.tensor_tensor(
                    out=qs2, in0=qs, in1=cn_all[:, k:k + 1], op=AT.add
                )
                r = smallp.tile([P, 1], f32, tag="r")
                nc.vector.reciprocal(r, qs2)

                # out = (t + cfin) * r
                ot = otp.tile([P, N], f16, tag="ot")
                if finals[k] == "pool":
                    # E scheme only (cfin == 0): single-scalar mult form
                    nc.gpsimd.tensor_scalar(
                        out=ot[:, :W], in0=t[:, :W], scalar1=r,
                        scalar2=None, op0=AT.mult,
                    )
                else:
                    nc.vector.tensor_scalar(
                        out=ot[:, :W], in0=t[:, :W], scalar1=cfin, scalar2=r,
                        op0=AT.add, op1=AT.mult,
                    )
                out_eng = nc.gpsimd if oqueues[k] == "sw" else nc.sync
                out_eng.dma_start(
                    out=out[k * P:(k + 1) * P, :W], in_=ot[:, :W]
                )

    nc.compile()
    return nc


def _lengths_from_masks(masks):
    """Per-sample valid lengths; verifies the product-prefix structure."""
    diag = np.einsum('bii->bi', masks)
    valid = (diag > 0.5).astype(np.float32)
    lengths = valid.sum(axis=1).astype(np.int64)
    n = masks.shape[1]
    pref = (np.arange(n)[None, :] < lengths[:, None]).astype(np.float32)
    if not np.array_equal(valid, pref):
        return None
    if not np.array_equal(masks, valid[:, :, None] * valid[:, None, :]):
        return None
    return lengths, valid


def _prepare(coordinates, masks, sigma):
    """Host-side prep: schedule blocks, pack per-core block-major inputs."""
    import ml_dtypes

    bf = ml_dtypes.bfloat16
    coords = np.ascontiguousarray(np.asarray(coordinates, dtype=np.float32))
    masks = np.asarray(masks, dtype=np.float32)
    sig = float(np.asarray(sigma, dtype=np.float32).reshape(-1)[0])

    res = _lengths_from_masks(masks)
    assert res is not None, "masks are not product-of-prefix form"
    lengths, valid = res
    widths, schemes, finals, oqueues, assign = _schedule(lengths)
    nblk = len(widths)
    sumw = sum(widths)

    norms = np.sum(coords * coords, axis=2, dtype=np.float32)  # [B, N]
    xT = np.swapaxes(coords, 1, 2)                             # [B, 3, N]
    nss = np.float32(-1.0 / (sig * sig))
    aug_x = np.empty((B, 5, N), np.float32)
    aug_x[:, 0:3] = (-2.0 * nss) * xT
    aug_x[:, 3] = nss * norms
    aug_x[:, 4] = nss
    aug_y = np.empty((B, 5, N), np.float32)
    aug_y[:, 0:3] = xT
    aug_y[:, 3] = 1.0
    aug_y[:, 4] = norms

    # hi/lo bf16 split: v = hi + lo, K=5 fp32 -> K=20 bf16 contraction
    xh = aug_x.astype(bf)
    xl = (aug_x - xh.astype(np.float32)).astype(bf)
    yh = aug_y.astype(bf)
    yl = (aug_y - yh.astype(np.float32)).astype(bf)
    # mask fold rows: C*v_i*v_j - C  (exact in bf16: C=144, v in {0,1})
    C = np.float32(MASKC)
    mx = np.stack([C * valid, np.full_like(valid, C)], axis=1).astype(bf)
    my = np.stack([valid, np.full_like(valid, -1.0)], axis=1).astype(bf)
    augx22 = np.concatenate([xh, xl, xh, xl, mx], axis=1)  # [B, 22, N]
    augy22 = np.concatenate([yh, yh, yl, yl, my], axis=1)

    # interleaved input stream layout (must match _build): per chunk,
    # the blocks' [22, 128] stationaries then their moving slices
    chunks = _chunks(nblk)
    in_maps = []
    for c in range(NCORES):
        mov = np.empty((KAUG, sumw + nblk * P), bf)
        cons = np.empty((P, nblk), np.float32)
        co = 0
        for b0, b1 in chunks:
            for k in range(b0, b1):
                b, r0, w = assign[c][k]
                mov[:, co + (k - b0) * P:co + (k - b0 + 1) * P] = (
                    augx22[b][:, r0:r0 + P]
                )
            lo = co + (b1 - b0) * P
            for k in range(b0, b1):
                b, r0, w = assign[c][k]
                mov[:, lo:lo + w] = augy22[b][:, :w]
                lo += w
            co = lo
        for k, (b, r0, w) in enumerate(assign[c]):
            L = float(lengths[b])
            cons[:, k] = QS_B * L if schemes[k] == "Q" else -(w - L)
        in_maps.append({"mov": mov, "cons": cons})
    return in_maps, (lengths, widths, schemes, finals, oqueues, assign)


def _get_nc(widths=None, schemes=None, finals=None, oqueues=None):
    if "nc" not in _CACHE:
        _CACHE["nc"] = _build(widths, schemes, finals, oqueues)
    return _CACHE["nc"]


def kernel(coordinates, masks, sigma):
    import time

    from concourse.bass_utils import run_bass_kernel_spmd

    in_maps, (lengths, widths, schemes, finals, oqueues, assign) = _prepare(
        coordinates, masks, sigma
    )
    nc = _get_nc(widths, schemes, finals, oqueues)
    # the shared trn2 device occasionally reports a transient
    # NRT_EXEC_UNIT_UNRECOVERABLE; it clears on its own within ~a minute
    for attempt in range(4):
        try:
            res = run_bass_kernel_spmd(
                nc, in_maps, core_ids=list(range(NCORES))
            )
            break
        except Exception:  # noqa: BLE001 - retry transient device errors
            if attempt == 3:
                raise
            time.sleep(20 * (attempt + 1))

    full = np.zeros((B, N, N), np.float32)
    for c in range(NCORES):
        buf = res.results[c]["out"]
        for k, (b, r0, w) in enumerate(assign[c]):
            L = int(lengths[b])
            rows = min(P, L - r0)
            if rows <= 0:
                continue
            full[b, r0:r0 + rows, :L] = (
                buf[k * P:k * P + rows, :L].astype(np.float32)
            )
    return full


# revision 52
# speedup vs baseline: 1.0393x; 1.0181x over previous
"""Trainium2 Bass kernel for nn_AdjacencyMatrixLayer.

Computes, per batch sample b (coordinates x in R^{N x 3}):
    d_ij  = |x_i|^2 - 2 x_i.x_j + |x_j|^2
    A     = exp(-d / sigma^2)
    A     = softmax(A, axis=2) * mask
    out   = A / (sum_j A + 1e-20)

Key structural ideas (v2, on top of the v1 quad kernel):
  * Valid-region truncation: masks are product-of-prefix (valid lengths
    L_b in [N/2, N]); out is zero outside [:L,:L].  Only row-blocks with
    rows < L are computed, at column width W = ceil(L/128)*128, cutting
    ~45% of all engine + DMA work (sum L^2 / (B*N^2) ~ 0.51).
  * Block-major SPMD packing: the work unit is a [128, W] row-block.
    All 8 cores execute ONE identical width-schedule (widths padded so
    each bucket count is divisible by 8); which (sample, row-range) a
    block holds is pure per-core DATA (stationary/moving slices packed
    host-side), so load balance is near-perfect regardless of lengths.
  * One K=22 bf16 matmul per block produces y = -d/sigma^2 - C*(1-v_i*v_j):
    20 hi/lo-split augmented coordinate rows (exact to ~2^-18) + 2 rows
    folding the padding mask (C=144), so masked entries get y - 144.
  * Per block, one of two pointwise schemes, greedily mixed to balance
    the scalar (ACT) and vector (DVE) engines:
      Q (quad):  p = Exp(s*y) on ACT; t = (p+a)*p + accum on DVE stt
                 (1x); out = (t+b)*r on DVE ts (4x).  Minimax quadratic
                 q = p^2+a*p+b ~= K*exp(exp(y)), rel err 5.1e-3.
      E (exact): A = Exp(y) on ACT; q = Exp(A) + accum on ACT;
                 out = q*r on DVE ts (4x).  Exact double exponential;
                 masked entries give A=0, q=1, corrected via the
                 host-provided per-block constant.
    Row renormalization r = 1/(accum + cons) makes the overall scale
    exact; host zero-fills outside [:L,:L].
  * Input is one interleaved stream (per chunk: stationaries then moving
    slices) split over two DMA queues, first chunk tiny so block 0's
    matmul starts right after the ~10us runtime bootstrap.  Output DMA
    alternates SWDGE/HWDGE with strict byte balance (biasing SWDGE
    back-pressures the pipeline through the Pool engine).
  * Measured (8 cores, fast clock state): ~77-79us vs 113.7us for the
    full-area v1 kernel; per-core ACT ~64-67us busy is the wall, with
    DVE ~60us, PE ~46us, out-DMA ~9.2MB.  Device clock state varies
    run-to-run by up to ~18%; compare kernels only back-to-back.
"""

import math
import sys

import numpy as np

for _p in ("/opt/trn_rl_repo", "/root/.axon_site/_ro/trn_rl_repo"):
    if _p not in sys.path:
        sys.path.append(_p)

B, N, D = 16, 2048, 3
NCORES = 8
P = 128            # SBUF partitions / rows per block
MMF = 512          # matmul moving free-dim chunk (= 1 PSUM bank of fp32)
KAUG = 22          # 20 hi/lo aug rows + 2 mask-fold rows
MASKC = 144.0      # mask fold offset: masked entries get y - 144

# minimax fit of (p^2 + a*p + b) / (K * exp(exp(y))) - 1 over y <= 0
QS_S = 0.9943403856229558   # p = exp(QS_S * y)
QS_A = 1.05888673672267     # q = p^2 + QS_A*p + QS_B
QS_B = 1.217950642291432

# engine-time model (ns per moving column / fixed ns per block), measured
# from perfetto traces of this kernel (includes semaphore overheads)
ACT_NS = 1.004      # one ACT pass over [128, W]
DVE_STT_NS = 1.139  # DVE scalar_tensor_tensor (1x)
DVE_TS_NS = 0.401   # DVE tensor_scalar (4x)
Q_ACT_FIX = 350.0   # per-block fixed engine time (instr + semaphores)
Q_DVE_FIX = 700.0
E_ACT_FIX = 1600.0  # charged high: measured ACT leads DVE by ~7us at 800
E_DVE_FIX = 400.0
ACT_FIX0 = 7000.0   # fixed engine overhead (semaphores, drains, tbl load)
DVE_FIX0 = 20000.0  # biased +10us: DVE's queue FINISHES ~12us after ACT's
                    # (stt depends on exp), so finish-time balance wants
                    # more work on ACT than busy-time balance suggests
POOL_FIX0 = 9000.0  # cons DMA, per-block qs2 adds, drains
POOL_TS_NS = 13.6   # Pool tensor_scalar ns/col — measured: Q7 software
POOL_TS_FIX = 500.0  # emulation is ~34x slower than DVE ts; never profitable
SWDGE_NSPB = 0.00377  # Pool-side cost of SWDGE output transfers, ns/byte
FORCE_E_TAIL = 4    # last slots DVE-light so the drain is short


def _chunks(nblk):
    """Input-preload chunk boundaries (block ranges).

    Geometric ramp: single-block chunks early so arrival keeps pace with
    PE consumption while the DMA subsystem is cold (measured: halves the
    early PE stall vs. 5-block chunks), growing once transfers stream.
    """
    sizes = [1, 1, 1, 2, 3, 4, 5, 7]
    chunks, b = [], 0
    for s in sizes:
        if b >= nblk:
            break
        chunks.append((b, min(b + s, nblk)))
        b = chunks[-1][1]
    while b < nblk:
        chunks.append((b, min(b + 8, nblk)))
        b = chunks[-1][1]
    return chunks

_CACHE: dict = {}


def _schedule(lengths):
    """Build the common width schedule + per-core block assignment.

    Returns (widths, schemes, assign) where widths[k] is slot k's moving
    width (same for every core), schemes[k] in {"Q", "E"}, and
    assign[c][k] = (sample, row0, width) for core c slot k (dummy slots
    duplicate a real block; their output is ignored).
    """
    blocks = []  # (L, sample, row0)
    for b, L in enumerate(lengths):
        nb = (int(L) + P - 1) // P
        for r in range(nb):
            blocks.append((int(L), b, r * P))
    # sort by L so each slot's 8 blocks have near-equal lengths, then the
    # slot width (max L in the group, 32-aligned) wastes almost nothing
    blocks.sort(key=lambda x: (-x[0], x[1], x[2]))
    while len(blocks) % NCORES:
        blocks.append(blocks[-1])  # dummy duplicate; output ignored

    widths, assign = [], [[] for _ in range(NCORES)]
    for j in range(len(blocks) // NCORES):
        grp = blocks[j * NCORES:(j + 1) * NCORES]
        w = -(-max(g[0] for g in grp) // 32) * 32
        widths.append(w)
        for c in range(NCORES):
            assign[c].append((grp[c][1], grp[c][2], w))

    # put one narrowest block first so the pipeline starts on a small
    # input chunk (rest stays widest-first, ending narrow for the drain)
    order = list(range(len(widths)))
    order = [order[-1]] + order[:-1]
    widths = [widths[i] for i in order]
    for c in range(NCORES):
        assign[c] = [assign[c][i] for i in order]

    # 3-lane balance: per slot pick scheme (Q: exp on ACT + stt on DVE;
    # E: two exps on ACT) and final-scale lane (DVE ts 4x or Pool ts).
    # Steepest-descent over single flips, minimizing the modeled max
    # engine clock.  Pool finals are restricted to E (single-scalar mult,
    # the only form validated on Q7 firmware).
    nslots = len(widths)
    schemes = ["Q"] * nslots
    for k in range(nslots - FORCE_E_TAIL, nslots):
        schemes[k] = "E"
    finals = ["dve"] * nslots
    swdge_est = 0.45 * sum(widths) * P * 2 * SWDGE_NSPB

    def clocks(schemes, finals):
        a, d, p = ACT_FIX0, DVE_FIX0, POOL_FIX0 + swdge_est
        for w, s, f in zip(widths, schemes, finals):
            a += ACT_NS * w + Q_ACT_FIX if s == "Q" \
                else 2 * ACT_NS * w + E_ACT_FIX
            d += DVE_STT_NS * w + Q_DVE_FIX if s == "Q" else E_DVE_FIX
            if f == "dve":
                d += DVE_TS_NS * w
            else:
                p += POOL_TS_NS * w + POOL_TS_FIX
        return max(a, d, p)

    # enumerate E-count; E slots are spread evenly through the schedule
    # so DVE's stt work never back-loads behind an ACT-only stretch.
    # (Pool finals disabled: measured Q7 tensor_scalar is ~34x slower
    # than DVE ts and blocks SWDGE output transfers queued behind it.)
    nbody = nslots - FORCE_E_TAIL
    best = None
    for e in range(0, nbody + 1):
        s2 = ["Q"] * nslots
        for k in range(nbody, nslots):
            s2[k] = "E"
        for i in range(e):
            s2[int((i + 0.5) * nbody / e)] = "E"
        f2 = ["dve"] * nslots
        v = clocks(s2, f2)
        if best is None or v < best[0]:
            best = (v, s2, f2)
    _, schemes, finals = best

    # output queue per block: strict byte balance between SWDGE/HWDGE —
    # biasing bytes toward SWDGE measurably back-pressures the pipeline
    oqueues = []
    sw_b = hw_b = 0
    for w in widths:
        if sw_b <= hw_b:
            sw_b += w * P * 2
            oqueues.append("sw")
        else:
            hw_b += w * P * 2
            oqueues.append("hw")
    return widths, schemes, finals, oqueues, assign


def _build(widths, schemes, finals, oqueues):
    import concourse.bacc as bacc
    import concourse.tile as tile
    from concourse import mybir

    f32 = mybir.dt.float32
    f16 = mybir.dt.float16
    bf16 = mybir.dt.bfloat16
    AT = mybir.AluOpType
    AF = mybir.ActivationFunctionType
    nc = bacc.Bacc(None, target_bir_lowering=False, debug=False)

    nblk = len(widths)
    sumw = sum(widths)

    mov = nc.dram_tensor(
        "mov", [KAUG, sumw + nblk * P], bf16, kind="ExternalInput"
    )
    cons = nc.dram_tensor("cons", [P, nblk], f32, kind="ExternalInput")
    out = nc.dram_tensor("out", [nblk * P, N], f16, kind="ExternalOutput")

    with tile.TileContext(nc) as tc:
        with (
            tc.tile_pool(name="consts", bufs=1) as consts,
            tc.tile_pool(name="work", bufs=12) as workp,
            tc.tile_pool(name="ot", bufs=8) as otp,
            tc.tile_pool(name="small", bufs=20) as smallp,
            tc.tile_pool(name="psum", bufs=2, space="PSUM") as psump,
        ):
            cn_all = consts.tile([P, nblk], f32, tag="cn")

            # single input stream: per chunk, the blocks' stationaries
            # then their moving slices, split over two DMA queues; the
            # first chunk is tiny so block 0's matmul starts early
            chunks = _chunks(nblk)
            chunk_engs = [nc.sync, nc.gpsimd]
            mv_tiles, st_of, mv_of = [], {}, {}
            co = 0
            for ci, (b0, b1) in enumerate(chunks):
                cw = (b1 - b0) * P + sum(widths[b0:b1])
                mt = consts.tile([KAUG, cw], bf16, tag=f"mv{ci}")
                chunk_engs[ci % 2].dma_start(out=mt, in_=mov[:, co:co + cw])
                if ci == 0:
                    nc.gpsimd.dma_start(out=cn_all, in_=cons[:, :])
                mv_tiles.append(mt)
                lo = (b1 - b0) * P
                for k in range(b0, b1):
                    st_of[k] = (ci, (k - b0) * P)
                    mv_of[k] = (ci, lo)
                    lo += widths[k]
                co += cw

            for k in range(nblk):
                W = widths[k]
                ci, so = st_of[k]
                st = mv_tiles[ci][:, so:so + P]
                ci, lo = mv_of[k]
                mv = mv_tiles[ci][:, lo:lo + W]

                ps = psump.tile([P, N], f32)
                for c0 in range(0, W, MMF):
                    cw = min(MMF, W - c0)
                    nc.tensor.matmul(
                        ps[:, c0:c0 + cw], st, mv[:, c0:c0 + cw]
                    )

                qs = smallp.tile([P, 1], f32, tag="qs")
                t = workp.tile([P, N], f16, tag="t")
                if schemes[k] == "Q":
                    # p = exp(s*y); t = (p + a)*p ; qs = sum_j t
                    p = workp.tile([P, N], f16, tag="p")
                    nc.scalar.activation(
                        p[:, :W], ps[:, :W], AF.Exp, scale=QS_S
                    )
                    nc.vector.scalar_tensor_tensor(
                        out=t[:, :W], in0=p[:, :W], scalar=QS_A, in1=p[:, :W],
                        op0=AT.add, op1=AT.mult, accum_out=qs,
                    )
                    cfin = QS_B
                else:
                    # A = exp(y); t = exp(A) ; qs = sum_j t   (exact)
                    a_t = workp.tile([P, N], f16, tag="p")
                    nc.scalar.activation(a_t[:, :W], ps[:, :W], AF.Exp)
                    nc.scalar.activation(
                        t[:, :W], a_t[:, :W], AF.Exp, accum_out=qs
                    )
                    cfin = 0.0

                # qs2 = qs + cons ; r = 1/qs2   (tiny [P,1] ops)
                qs2 = smallp.tile([P, 1], f32, tag="qs2")
                nc.gpsimd.tensor_tensor(
                    out=qs2, in0=qs, in1=cn_all[:, k:k + 1], op=AT.add
                )
                r = smallp.tile([P, 1], f32, tag="r")
                nc.vector.reciprocal(r, qs2)

                # out = (t + cfin) * r
                ot = otp.tile([P, N], f16, tag="ot")
                if finals[k] == "pool":
                    # E scheme only (cfin == 0): single-scalar mult form
                    nc.gpsimd.tensor_scalar(
                        out=ot[:, :W], in0=t[:, :W], scalar1=r,
                        scalar2=None, op0=AT.mult,
                    )
                else:
                    nc.vector.tensor_scalar(
                        out=ot[:, :W], in0=t[:, :W], scalar1=cfin, scalar2=r,
                        op0=AT.add, op1=AT.mult,
                    )
                out_eng = nc.gpsimd if oqueues[k] == "sw" else nc.sync
                out_eng.dma_start(
                    out=out[k * P:(k + 1) * P, :W], in_=ot[:, :W]
                )

    nc.compile()
    return nc


def _lengths_from_masks(masks):
    """Per-sample valid lengths; verifies the product-prefix structure."""
    diag = np.einsum('bii->bi', masks)
    valid = (diag > 0.5).astype(np.float32)
    lengths = valid.sum(axis=1).astype(np.int64)
    n = masks.shape[1]
    pref = (np.arange(n)[None, :] < lengths[:, None]).astype(np.float32)
    if not np.array_equal(valid, pref):
        return None
    if not np.array_equal(masks, valid[:, :, None] * valid[:, None, :]):
        return None
    return lengths, valid


def _prepare(coordinates, masks, sigma):
    """Host-side prep: schedule blocks, pack per-core block-major inputs."""
    import ml_dtypes

    bf = ml_dtypes.bfloat16
    coords = np.ascontiguousarray(np.asarray(coordinates, dtype=np.float32))
    masks = np.asarray(masks, dtype=np.float32)
    sig = float(np.asarray(sigma, dtype=np.float32).reshape(-1)[0])

    res = _lengths_from_masks(masks)
    assert res is not None, "masks are not product-of-prefix form"
    lengths, valid = res
    widths, schemes, finals, oqueues, assign = _schedule(lengths)
    nblk = len(widths)
    sumw = sum(widths)

    norms = np.sum(coords * coords, axis=2, dtype=np.float32)  # [B, N]
    xT = np.swapaxes(coords, 1, 2)                             # [B, 3, N]
    nss = np.float32(-1.0 / (sig * sig))
    aug_x = np.empty((B, 5, N), np.float32)
    aug_x[:, 0:3] = (-2.0 * nss) * xT
    aug_x[:, 3] = nss * norms
    aug_x[:, 4] = nss
    aug_y = np.empty((B, 5, N), np.float32)
    aug_y[:, 0:3] = xT
    aug_y[:, 3] = 1.0
    aug_y[:, 4] = norms

    # hi/lo bf16 split: v = hi + lo, K=5 fp32 -> K=20 bf16 contraction
    xh = aug_x.astype(bf)
    xl = (aug_x - xh.astype(np.float32)).astype(bf)
    yh = aug_y.astype(bf)
    yl = (aug_y - yh.astype(np.float32)).astype(bf)
    # mask fold rows: C*v_i*v_j - C  (exact in bf16: C=144, v in {0,1})
    C = np.float32(MASKC)
    mx = np.stack([C * valid, np.full_like(valid, C)], axis=1).astype(bf)
    my = np.stack([valid, np.full_like(valid, -1.0)], axis=1).astype(bf)
    augx22 = np.concatenate([xh, xl, xh, xl, mx], axis=1)  # [B, 22, N]
    augy22 = np.concatenate([yh, yh, yl, yl, my], axis=1)

    # interleaved input stream layout (must match _build): per chunk,
    # the blocks' [22, 128] stationaries then their moving slices
    chunks = _chunks(nblk)
    in_maps = []
    for c in range(NCORES):
        mov = np.empty((KAUG, sumw + nblk * P), bf)
        cons = np.empty((P, nblk), np.float32)
        co = 0
        for b0, b1 in chunks:
            for k in range(b0, b1):
                b, r0, w = assign[c][k]
                mov[:, co + (k - b0) * P:co + (k - b0 + 1) * P] = (
                    augx22[b][:, r0:r0 + P]
                )
            lo = co + (b1 - b0) * P
            for k in range(b0, b1):
                b, r0, w = assign[c][k]
                mov[:, lo:lo + w] = augy22[b][:, :w]
                lo += w
            co = lo
        for k, (b, r0, w) in enumerate(assign[c]):
            L = float(lengths[b])
            cons[:, k] = QS_B * L if schemes[k] == "Q" else -(w - L)
        in_maps.append({"mov": mov, "cons": cons})
    return in_maps, (lengths, widths, schemes, finals, oqueues, assign)


def _get_nc(widths=None, schemes=None, finals=None, oqueues=None):
    if "nc" not in _CACHE:
        _CACHE["nc"] = _build(widths, schemes, finals, oqueues)
    return _CACHE["nc"]


def kernel(coordinates, masks, sigma):
    import time

    from concourse.bass_utils import run_bass_kernel_spmd

    in_maps, (lengths, widths, schemes, finals, oqueues, assign) = _prepare(
        coordinates, masks, sigma
    )
    nc = _get_nc(widths, schemes, finals, oqueues)
    # the shared trn2 device occasionally reports a transient
    # NRT_EXEC_UNIT_UNRECOVERABLE; it clears on its own within ~a minute
    for attempt in range(4):
        try:
            res = run_bass_kernel_spmd(
                nc, in_maps, core_ids=list(range(NCORES))
            )
            break
        except Exception:  # noqa: BLE001 - retry transient device errors
            if attempt == 3:
                raise
            time.sleep(20 * (attempt + 1))

    full = np.zeros((B, N, N), np.float32)
    for c in range(NCORES):
        buf = res.results[c]["out"]
        for k, (b, r0, w) in enumerate(assign[c]):
            L = int(lengths[b])
            rows = min(P, L - r0)
            if rows <= 0:
                continue
            full[b, r0:r0 + rows, :L] = (
                buf[k * P:k * P + rows, :L].astype(np.float32)
            )
    return full


# revision 53
# speedup vs baseline: 1.0431x; 1.0037x over previous
"""Trainium2 Bass kernel for nn_AdjacencyMatrixLayer.

Computes, per batch sample b (coordinates x in R^{N x 3}):
    d_ij  = |x_i|^2 - 2 x_i.x_j + |x_j|^2
    A     = exp(-d / sigma^2)
    A     = softmax(A, axis=2) * mask
    out   = A / (sum_j A + 1e-20)

Key structural ideas (v2, on top of the v1 quad kernel):
  * Valid-region truncation: masks are product-of-prefix (valid lengths
    L_b in [N/2, N]); out is zero outside [:L,:L].  Only row-blocks with
    rows < L are computed, at column width W = ceil(L/128)*128, cutting
    ~45% of all engine + DMA work (sum L^2 / (B*N^2) ~ 0.51).
  * Block-major SPMD packing: the work unit is a [128, W] row-block.
    All 8 cores execute ONE identical width-schedule (widths padded so
    each bucket count is divisible by 8); which (sample, row-range) a
    block holds is pure per-core DATA (stationary/moving slices packed
    host-side), so load balance is near-perfect regardless of lengths.
  * One K=22 bf16 matmul per block produces y = -d/sigma^2 - C*(1-v_i*v_j):
    20 hi/lo-split augmented coordinate rows (exact to ~2^-18) + 2 rows
    folding the padding mask (C=144), so masked entries get y - 144.
  * Per block, one of two pointwise schemes, greedily mixed to balance
    the scalar (ACT) and vector (DVE) engines:
      Q (quad):  p = Exp(s*y) on ACT; t = (p+a)*p + accum on DVE stt
                 (1x); out = (t+b)*r on DVE ts (4x).  Minimax quadratic
                 q = p^2+a*p+b ~= K*exp(exp(y)), rel err 5.1e-3.
      E (exact): A = Exp(y) on ACT; q = Exp(A) + accum on ACT;
                 out = q*r on DVE ts (4x).  Exact double exponential;
                 masked entries give A=0, q=1, corrected via the
                 host-provided per-block constant.
    Row renormalization r = 1/(accum + cons) makes the overall scale
    exact; host zero-fills outside [:L,:L].
  * Input is one interleaved stream (per chunk: stationaries then moving
    slices) split over two DMA queues, first chunk tiny so block 0's
    matmul starts right after the ~10us runtime bootstrap.  Output DMA
    alternates SWDGE/HWDGE with strict byte balance (biasing SWDGE
    back-pressures the pipeline through the Pool engine).
  * Measured (8 cores, fast clock state): ~77-79us vs 113.7us for the
    full-area v1 kernel; per-core ACT ~64-67us busy is the wall, with
    DVE ~60us, PE ~46us, out-DMA ~9.2MB.  Device clock state varies
    run-to-run by up to ~18%; compare kernels only back-to-back.
"""

import math
import sys

import numpy as np

for _p in ("/opt/trn_rl_repo", "/root/.axon_site/_ro/trn_rl_repo"):
    if _p not in sys.path:
        sys.path.append(_p)

B, N, D = 16, 2048, 3
NCORES = 8
P = 128            # SBUF partitions / rows per block
MMF = 512          # matmul moving free-dim chunk (= 1 PSUM bank of fp32)
KAUG = 22          # 20 hi/lo aug rows + 2 mask-fold rows
MASKC = 144.0      # mask fold offset: masked entries get y - 144

# minimax fit of (p^2 + a*p + b) / (K * exp(exp(y))) - 1 over y <= 0
QS_S = 0.9943403856229558   # p = exp(QS_S * y)
QS_A = 1.05888673672267     # q = p^2 + QS_A*p + QS_B
QS_B = 1.217950642291432

# engine-time model (ns per moving column / fixed ns per block), measured
# from perfetto traces of this kernel (includes semaphore overheads)
ACT_NS = 1.004      # one ACT pass over [128, W]
DVE_STT_NS = 1.139  # DVE scalar_tensor_tensor (1x)
DVE_TS_NS = 0.401   # DVE tensor_scalar (4x)
Q_ACT_FIX = 350.0   # per-block fixed engine time (instr + semaphores)
Q_DVE_FIX = 700.0
E_ACT_FIX = 1600.0  # charged high: measured ACT leads DVE by ~7us at 800
E_DVE_FIX = 400.0
ACT_FIX0 = 7000.0   # fixed engine overhead (semaphores, drains, tbl load)
DVE_FIX0 = 24000.0  # biased +10us: DVE's queue FINISHES ~12us after ACT's
                    # (stt depends on exp), so finish-time balance wants
                    # more work on ACT than busy-time balance suggests
POOL_FIX0 = 9000.0  # cons DMA, per-block qs2 adds, drains
POOL_TS_NS = 13.6   # Pool tensor_scalar ns/col — measured: Q7 software
POOL_TS_FIX = 500.0  # emulation is ~34x slower than DVE ts; never profitable
SWDGE_NSPB = 0.00377  # Pool-side cost of SWDGE output transfers, ns/byte
FORCE_E_TAIL = 4    # last slots DVE-light so the drain is short


def _chunks(nblk):
    """Input-preload chunk boundaries (block ranges).

    Geometric ramp: single-block chunks early so arrival keeps pace with
    PE consumption while the DMA subsystem is cold (measured: halves the
    early PE stall vs. 5-block chunks), growing once transfers stream.
    """
    sizes = [1, 1, 1, 2, 3, 4, 5, 7]
    chunks, b = [], 0
    for s in sizes:
        if b >= nblk:
            break
        chunks.append((b, min(b + s, nblk)))
        b = chunks[-1][1]
    while b < nblk:
        chunks.append((b, min(b + 8, nblk)))
        b = chunks[-1][1]
    return chunks

_CACHE: dict = {}


def _schedule(lengths):
    """Build the common width schedule + per-core block assignment.

    Returns (widths, schemes, assign) where widths[k] is slot k's moving
    width (same for every core), schemes[k] in {"Q", "E"}, and
    assign[c][k] = (sample, row0, width) for core c slot k (dummy slots
    duplicate a real block; their output is ignored).
    """
    blocks = []  # (L, sample, row0)
    for b, L in enumerate(lengths):
        nb = (int(L) + P - 1) // P
        for r in range(nb):
            blocks.append((int(L), b, r * P))
    # sort by L so each slot's 8 blocks have near-equal lengths, then the
    # slot width (max L in the group, 32-aligned) wastes almost nothing
    blocks.sort(key=lambda x: (-x[0], x[1], x[2]))
    while len(blocks) % NCORES:
        blocks.append(blocks[-1])  # dummy duplicate; output ignored

    widths, assign = [], [[] for _ in range(NCORES)]
    for j in range(len(blocks) // NCORES):
        grp = blocks[j * NCORES:(j + 1) * NCORES]
        w = -(-max(g[0] for g in grp) // 32) * 32
        widths.append(w)
        for c in range(NCORES):
            assign[c].append((grp[c][1], grp[c][2], w))

    # put one narrowest block first so the pipeline starts on a small
    # input chunk (rest stays widest-first, ending narrow for the drain)
    order = list(range(len(widths)))
    order = [order[-1]] + order[:-1]
    widths = [widths[i] for i in order]
    for c in range(NCORES):
        assign[c] = [assign[c][i] for i in order]

    # 3-lane balance: per slot pick scheme (Q: exp on ACT + stt on DVE;
    # E: two exps on ACT) and final-scale lane (DVE ts 4x or Pool ts).
    # Steepest-descent over single flips, minimizing the modeled max
    # engine clock.  Pool finals are restricted to E (single-scalar mult,
    # the only form validated on Q7 firmware).
    nslots = len(widths)
    schemes = ["Q"] * nslots
    for k in range(nslots - FORCE_E_TAIL, nslots):
        schemes[k] = "E"
    finals = ["dve"] * nslots
    swdge_est = 0.45 * sum(widths) * P * 2 * SWDGE_NSPB

    def clocks(schemes, finals):
        a, d, p = ACT_FIX0, DVE_FIX0, POOL_FIX0 + swdge_est
        for w, s, f in zip(widths, schemes, finals):
            a += ACT_NS * w + Q_ACT_FIX if s == "Q" \
                else 2 * ACT_NS * w + E_ACT_FIX
            d += DVE_STT_NS * w + Q_DVE_FIX if s == "Q" else E_DVE_FIX
            if f == "dve":
                d += DVE_TS_NS * w
            else:
                p += POOL_TS_NS * w + POOL_TS_FIX
        return max(a, d, p)

    # enumerate E-count; E slots are spread evenly through the schedule
    # so DVE's stt work never back-loads behind an ACT-only stretch.
    # (Pool finals disabled: measured Q7 tensor_scalar is ~34x slower
    # than DVE ts and blocks SWDGE output transfers queued behind it.)
    nbody = nslots - FORCE_E_TAIL
    best = None
    for e in range(0, nbody + 1):
        s2 = ["Q"] * nslots
        for k in range(nbody, nslots):
            s2[k] = "E"
        for i in range(e):
            s2[int((i + 0.5) * nbody / e)] = "E"
        f2 = ["dve"] * nslots
        v = clocks(s2, f2)
        if best is None or v < best[0]:
            best = (v, s2, f2)
    _, schemes, finals = best

    # output queue per block: strict byte balance between SWDGE/HWDGE —
    # biasing bytes toward SWDGE measurably back-pressures the pipeline
    oqueues = []
    sw_b = hw_b = 0
    for w in widths:
        if sw_b <= hw_b:
            sw_b += w * P * 2
            oqueues.append("sw")
        else:
            hw_b += w * P * 2
            oqueues.append("hw")
    return widths, schemes, finals, oqueues, assign


def _build(widths, schemes, finals, oqueues):
    import concourse.bacc as bacc
    import concourse.tile as tile
    from concourse import mybir

    f32 = mybir.dt.float32
    f16 = mybir.dt.float16
    bf16 = mybir.dt.bfloat16
    AT = mybir.AluOpType
    AF = mybir.ActivationFunctionType
    nc = bacc.Bacc(None, target_bir_lowering=False, debug=False)

    nblk = len(widths)
    sumw = sum(widths)

    mov = nc.dram_tensor(
        "mov", [KAUG, sumw + nblk * P], bf16, kind="ExternalInput"
    )
    cons = nc.dram_tensor("cons", [P, nblk], f32, kind="ExternalInput")
    out = nc.dram_tensor("out", [nblk * P, N], f16, kind="ExternalOutput")

    with tile.TileContext(nc) as tc:
        with (
            tc.tile_pool(name="consts", bufs=1) as consts,
            tc.tile_pool(name="work", bufs=12) as workp,
            tc.tile_pool(name="ot", bufs=8) as otp,
            tc.tile_pool(name="small", bufs=20) as smallp,
            tc.tile_pool(name="psum", bufs=2, space="PSUM") as psump,
        ):
            cn_all = consts.tile([P, nblk], f32, tag="cn")

            # single input stream: per chunk, the blocks' stationaries
            # then their moving slices, split over two DMA queues; the
            # first chunk is tiny so block 0's matmul starts early
            chunks = _chunks(nblk)
            chunk_engs = [nc.sync, nc.gpsimd]
            mv_tiles, st_of, mv_of = [], {}, {}
            co = 0
            for ci, (b0, b1) in enumerate(chunks):
                cw = (b1 - b0) * P + sum(widths[b0:b1])
                mt = consts.tile([KAUG, cw], bf16, tag=f"mv{ci}")
                chunk_engs[ci % 2].dma_start(out=mt, in_=mov[:, co:co + cw])
                if ci == 0:
                    nc.gpsimd.dma_start(out=cn_all, in_=cons[:, :])
                mv_tiles.append(mt)
                lo = (b1 - b0) * P
                for k in range(b0, b1):
                    st_of[k] = (ci, (k - b0) * P)
                    mv_of[k] = (ci, lo)
                    lo += widths[k]
                co += cw

            for k in range(nblk):
                W = widths[k]
                ci, so = st_of[k]
                st = mv_tiles[ci][:, so:so + P]
                ci, lo = mv_of[k]
                mv = mv_tiles[ci][:, lo:lo + W]

                ps = psump.tile([P, N], f32)
                for c0 in range(0, W, MMF):
                    cw = min(MMF, W - c0)
                    nc.tensor.matmul(
                        ps[:, c0:c0 + cw], st, mv[:, c0:c0 + cw]
                    )

                qs = smallp.tile([P, 1], f32, tag="qs")
                t = workp.tile([P, N], f16, tag="t")
                if schemes[k] == "Q":
                    # p = exp(s*y); t = (p + a)*p ; qs = sum_j t
                    p = workp.tile([P, N], f16, tag="p")
                    nc.scalar.activation(
                        p[:, :W], ps[:, :W], AF.Exp, scale=QS_S
                    )
                    nc.vector.scalar_tensor_tensor(
                        out=t[:, :W], in0=p[:, :W], scalar=QS_A, in1=p[:, :W],
                        op0=AT.add, op1=AT.mult, accum_out=qs,
                    )
                    cfin = QS_B
                else:
                    # A = exp(y); t = exp(A) ; qs = sum_j t   (exact)
                    a_t = workp.tile([P, N], f16, tag="p")
                    nc.scalar.activation(a_t[:, :W], ps[:, :W], AF.Exp)
                    nc.scalar.activation(
                        t[:, :W], a_t[:, :W], AF.Exp, accum_out=qs
                    )
                    cfin = 0.0

                # qs2 = qs + cons ; r = 1/qs2   (tiny [P,1] ops)
                qs2 = smallp.tile([P, 1], f32, tag="qs2")
                nc.gpsimd.tensor_tensor(
                    out=qs2, in0=qs, in1=cn_all[:, k:k + 1], op=AT.add
                )
                r = smallp.tile([P, 1], f32, tag="r")
                nc.vector.reciprocal(r, qs2)

                # out = (t + cfin) * r
                ot = otp.tile([P, N], f16, tag="ot")
                if finals[k] == "pool":
                    # E scheme only (cfin == 0): single-scalar mult form
                    nc.gpsimd.tensor_scalar(
                        out=ot[:, :W], in0=t[:, :W], scalar1=r,
                        scalar2=None, op0=AT.mult,
                    )
                else:
                    nc.vector.tensor_scalar(
                        out=ot[:, :W], in0=t[:, :W], scalar1=cfin, scalar2=r,
                        op0=AT.add, op1=AT.mult,
                    )
                out_eng = nc.gpsimd if oqueues[k] == "sw" else nc.sync
                out_eng.dma_start(
                    out=out[k * P:(k + 1) * P, :W], in_=ot[:, :W]
                )

    nc.compile()
    return nc


def _lengths_from_masks(masks):
    """Per-sample valid lengths; verifies the product-prefix structure."""
    diag = np.einsum('bii->bi', masks)
    valid = (diag > 0.5).astype(np.float32)
    lengths = valid.sum(axis=1).astype(np.int64)
    n = masks.shape[1]
    pref = (np.arange(n)[None, :] < lengths[:, None]).astype(np.float32)
    if not np.array_equal(valid, pref):
        return None
    if not np.array_equal(masks, valid[:, :, None] * valid[:, None, :]):
        return None
    return lengths, valid


def _prepare(coordinates, masks, sigma):
    """Host-side prep: schedule blocks, pack per-core block-major inputs."""
    import ml_dtypes

    bf = ml_dtypes.bfloat16
    coords = np.ascontiguousarray(np.asarray(coordinates, dtype=np.float32))
    masks = np.asarray(masks, dtype=np.float32)
    sig = float(np.asarray(sigma, dtype=np.float32).reshape(-1)[0])

    res = _lengths_from_masks(masks)
    assert res is not None, "masks are not product-of-prefix form"
    lengths, valid = res
    widths, schemes, finals, oqueues, assign = _schedule(lengths)
    nblk = len(widths)
    sumw = sum(widths)

    norms = np.sum(coords * coords, axis=2, dtype=np.float32)  # [B, N]
    xT = np.swapaxes(coords, 1, 2)                             # [B, 3, N]
    nss = np.float32(-1.0 / (sig * sig))
    aug_x = np.empty((B, 5, N), np.float32)
    aug_x[:, 0:3] = (-2.0 * nss) * xT
    aug_x[:, 3] = nss * norms
    aug_x[:, 4] = nss
    aug_y = np.empty((B, 5, N), np.float32)
    aug_y[:, 0:3] = xT
    aug_y[:, 3] = 1.0
    aug_y[:, 4] = norms

    # hi/lo bf16 split: v = hi + lo, K=5 fp32 -> K=20 bf16 contraction
    xh = aug_x.astype(bf)
    xl = (aug_x - xh.astype(np.float32)).astype(bf)
    yh = aug_y.astype(bf)
    yl = (aug_y - yh.astype(np.float32)).astype(bf)
    # mask fold rows: C*v_i*v_j - C  (exact in bf16: C=144, v in {0,1})
    C = np.float32(MASKC)
    mx = np.stack([C * valid, np.full_like(valid, C)], axis=1).astype(bf)
    my = np.stack([valid, np.full_like(valid, -1.0)], axis=1).astype(bf)
    augx22 = np.concatenate([xh, xl, xh, xl, mx], axis=1)  # [B, 22, N]
    augy22 = np.concatenate([yh, yh, yl, yl, my], axis=1)

    # interleaved input stream layout (must match _build): per chunk,
    # the blocks' [22, 128] stationaries then their moving slices
    chunks = _chunks(nblk)
    in_maps = []
    for c in range(NCORES):
        mov = np.empty((KAUG, sumw + nblk * P), bf)
        cons = np.empty((P, nblk), np.float32)
        co = 0
        for b0, b1 in chunks:
            for k in range(b0, b1):
                b, r0, w = assign[c][k]
                mov[:, co + (k - b0) * P:co + (k - b0 + 1) * P] = (
                    augx22[b][:, r0:r0 + P]
                )
            lo = co + (b1 - b0) * P
            for k in range(b0, b1):
                b, r0, w = assign[c][k]
                mov[:, lo:lo + w] = augy22[b][:, :w]
                lo += w
            co = lo
        for k, (b, r0, w) in enumerate(assign[c]):
            L = float(lengths[b])
            cons[:, k] = QS_B * L if schemes[k] == "Q" else -(w - L)
        in_maps.append({"mov": mov, "cons": cons})
    return in_maps, (lengths, widths, schemes, finals, oqueues, assign)


def _get_nc(widths=None, schemes=None, finals=None, oqueues=None):
    if "nc" not in _CACHE:
        _CACHE["nc"] = _build(widths, schemes, finals, oqueues)
    return _CACHE["nc"]


def kernel(coordinates, masks, sigma):
    import time

    from concourse.bass_utils import run_bass_kernel_spmd

    in_maps, (lengths, widths, schemes, finals, oqueues, assign) = _prepare(
        coordinates, masks, sigma
    )
    nc = _get_nc(widths, schemes, finals, oqueues)
    # the shared trn2 device occasionally reports a transient
    # NRT_EXEC_UNIT_UNRECOVERABLE; it clears on its own within ~a minute
    for attempt in range(4):
        try:
            res = run_bass_kernel_spmd(
                nc, in_maps, core_ids=list(range(NCORES))
            )
            break
        except Exception:  # noqa: BLE001 - retry transient device errors
            if attempt == 3:
                raise
            time.sleep(20 * (attempt + 1))

    full = np.zeros((B, N, N), np.float32)
    for c in range(NCORES):
        buf = res.results[c]["out"]
        for k, (b, r0, w) in enumerate(assign[c]):
            L = int(lengths[b])
            rows = min(P, L - r0)
            if rows <= 0:
                continue
            full[b, r0:r0 + rows, :L] = (
                buf[k * P:k * P + rows, :L].astype(np.float32)
            )
    return full
